# revision 2
# baseline (speedup 1.0000x reference)
"""Trainium2 Bass kernel for nn_CSATransformer_25778393710760.

Math: with the given parameters (all biases zero, ln identity) the module
reduces exactly to
    out = LN(relu(x @ pfn_w1) @ pfn_w2 + x)
(LayerNorm cancels the positive per-row colsum scale; see kernel_baseline).

Device kernel (per core, one batch example, L=4096 rows, D=128):
  - x is read ONCE from HBM, already transposed into SBUF via per-chunk
    xbar DMA transposes (bf16).  No row-major copy of x is needed: the
    residual (and its row-centering) is folded into the PE as
    po[:,c,:] = xT_c @ C + relu(x @ W1)_c @ (W2 C),  C = I - J/128,
    so po = y - rowmean(y) exactly and LN reduces to po * rsqrt(mean(po^2)+eps).
  - row mapping: row = 32p + 16Q + c (p = partition, quad Q, chunk c=0..15).
  - per quad (2048 rows) one PSUM tile [128,16,128] f32 (4 banks) holds both
    the mm1 staging (in the opposite pair's banks) and the final po chunks.
  - LN tail: ACT Square (1 op) -> DVE reduce -> ACT sqrt -> DVE recip ->
    DVE multiply (og, bf16) -> HWDGE store (bf16; host upcasts to fp32).
"""

import os
import numpy as np

B, L, DX = 8, 4096, 128
_NQ = 2            # quads per core
_C = 16            # chunks (rows per partition) per quad

_prog_cache = {}


def _build_program():
    import concourse.tile as tile
    from concourse import bacc, mybir
    from concourse.bass import ts

    f32 = mybir.dt.float32
    bf16 = mybir.dt.bfloat16
    AF = mybir.ActivationFunctionType
    OP = mybir.AluOpType
    AX = mybir.AxisListType

    nc = bacc.Bacc(None, target_bir_lowering=False)
    xb = nc.dram_tensor("xb", [L, DX], bf16, kind="ExternalInput")
    wpack = nc.dram_tensor("wpack", [DX, 3 * DX], bf16, kind="ExternalInput")
    y = nc.dram_tensor("y", [L, DX], bf16, kind="ExternalOutput")

    # row = 32*p + k  (k = 16*Q + c)
    xb_r = xb.rearrange("(p k) d -> p k d", p=128)
    y_r = y.rearrange("(p k) d -> p k d", p=128)

    with tile.TileContext(nc) as tc:
        with (
            tc.tile_pool(name="consts", bufs=1) as consts,
            tc.tile_pool(name="xt", bufs=2) as xt_pool,
            tc.tile_pool(name="y1s", bufs=2) as y1s_pool,
            tc.tile_pool(name="sqq", bufs=2) as sqq_pool,
            tc.tile_pool(name="og", bufs=2) as og_pool,
            tc.tile_pool(name="small", bufs=4) as small,
            tc.tile_pool(name="ps", bufs=2, space="PSUM") as ps,
        ):
            # ---- weights first on the scalar ring: gate matmuls
            wp = consts.tile([128, 3 * DX], bf16)
            nc.scalar.dma_start(out=wp, in_=wpack[:, :])
            w1_sb = wp[:, 0:128]
            w2c_sb = wp[:, 128:256]
            cmat_sb = wp[:, 256:384]

            eps = consts.tile([128, 1], f32)
            nc.vector.memset(eps, 1e-6)

            # ---- ACT table warms (Relu/Square/Sqrt) before data lands
            warm = consts.tile([128, 1], f32)
            nc.scalar.activation(out=warm, in_=eps, func=AF.Relu)
            nc.scalar.activation(out=warm, in_=eps, func=AF.Square)
            nc.scalar.activation(out=warm, in_=eps, func=AF.Sqrt, bias=eps)

            # ---- PE HAM warmup: real bf16 matmuls on the weight pack
            pewarm = ps.tile([128, _C, 128], f32, tag="T")
            for _ in range(4):
                nc.tensor.matmul(
                    pewarm[:, 0:3, :].rearrange("p c d -> p (c d)"),
                    lhsT=cmat_sb, rhs=wp[:, :], start=True, stop=True,
                )
            warmsink = consts.tile([128, 1], f32)
            nc.vector.tensor_copy(out=warmsink, in_=pewarm[:, 0, 0:1])

            for q in range(_NQ):
                # ---- load x transposed: xT[d, c, p] = x[32p + 16q + c, d]
                xT = xt_pool.tile([128, _C, 128], bf16, tag="xT")
                for c in range(_C):
                    nc.sync.dma_start_transpose(
                        out=xT[:, c, :], in_=xb_r[:, _C * q + c, :]
                    )

                # ---- quad PSUM tile: mm1 stages y1 in the opposite pair's
                # banks, mm2 overwrites with po chunks.
                T = ps.tile([128, _C, 128], f32, tag="T")
                # y1 for chunk c lands at slot (8 + c) % 16
                for h in range(4):  # rhs chunk groups 0:4, 4:8, 8:12, 12:16
                    slot = (8 + 4 * h) % 16
                    nc.tensor.matmul(
                        T[:, slot : slot + 4, :].rearrange("p c d -> p (c d)"),
                        lhsT=w1_sb,
                        rhs=xT[:, ts(h, 4), :].rearrange("d c p -> d (c p)"),
                        start=True, stop=True,
                    )

                # ---- relu on the whole quad (y1 occupies all 16 slots)
                y1s = y1s_pool.tile([128, _C, 128], bf16, tag="y1s")
                nc.scalar.activation(
                    out=y1s.rearrange("e c p -> e (c p)"),
                    in_=T.rearrange("e c p -> e (c p)"),
                    func=AF.Relu,
                )

                # ---- po chunks: xc + relu@W2C, accumulated per chunk
                for c in range(_C):
                    nc.tensor.matmul(
                        T[:, c, :], lhsT=xT[:, c, :], rhs=cmat_sb,
                        start=True, stop=False,
                    )
                    nc.tensor.matmul(
                        T[:, c, :], lhsT=y1s[:, (8 + c) % 16, :], rhs=w2c_sb,
                        start=False, stop=True,
                    )

                # ---- LN tail: mean(po)=0 by construction
                sqq = sqq_pool.tile([128, _C, 128], bf16, tag="sqq")
                nc.scalar.activation(
                    out=sqq.rearrange("p c d -> p (c d)"),
                    in_=T.rearrange("p c d -> p (c d)"),
                    func=AF.Square,
                )
                ssq = small.tile([128, _C], f32, tag="ssq")
                nc.vector.tensor_reduce(out=ssq, in_=sqq, axis=AX.X, op=OP.add)
                std = small.tile([128, _C], f32, tag="std")
                nc.scalar.activation(out=std, in_=ssq, func=AF.Sqrt,
                                     scale=1.0 / 128.0, bias=eps)
                rstd = small.tile([128, _C], f32, tag="rstd")
                nc.vector.reciprocal(out=rstd, in_=std)

                og = og_pool.tile([128, _C, 128], bf16, tag="og")
                rb = rstd.to_broadcast([128, _C, 128])
                nc.vector.tensor_tensor(out=og, in0=T, in1=rb, op=OP.mult)

                nc.sync.dma_start(out=y_r[:, ts(q, _C), :], in_=og)
    nc.finalize()
    return nc


def _ensure_ntff_hook():
    """Register the axon NTFF profiling hook if the image lacks antenv.axon_hooks."""
    try:
        from antenv.axon_hooks import get_axon_ntff_profile_hook  # noqa: F401
        return
    except ImportError:
        pass
    import sys
    import types

    import antenv
    from trn_agent_boot.trn_boot import _ntff_profile_via_ctypes

    hook = _ntff_profile_via_ctypes("/opt/axon/libaxon_pjrt.so")
    mod = types.ModuleType("antenv.axon_hooks")
    mod._hook = hook
    mod.set_axon_ntff_profile_hook = lambda h: setattr(mod, "_hook", h)
    mod.get_axon_ntff_profile_hook = lambda: mod._hook
    sys.modules["antenv.axon_hooks"] = mod
    antenv.axon_hooks = mod


def _run_device(x, w1, w2, trace=False):
    import ml_dtypes
    import concourse.bass_utils as bass_utils
    from concourse.bass_utils import run_bass_kernel_spmd

    if trace:
        try:
            _ensure_ntff_hook()
            bass_utils.upload_artifacts = lambda tmpdir: str(tmpdir)
        except Exception as e:  # profiling is best-effort
            print(f"ntff hook unavailable ({e}); running without trace")
            trace = False

    if "prog" not in _prog_cache:
        _prog_cache["prog"] = _build_program()
    nc = _prog_cache["prog"]

    bf = ml_dtypes.bfloat16
    x = np.ascontiguousarray(x, dtype=np.float32)
    xb16 = x.astype(bf)

    w1c = np.ascontiguousarray(w1, dtype=np.float32)
    w2c = np.ascontiguousarray(w2, dtype=np.float32)
    cmat = np.eye(DX, dtype=np.float32) - np.float32(1.0 / DX)
    w2cc = (w2c @ cmat).astype(bf)
    wpack = np.concatenate(
        [w1c.astype(bf), w2cc, cmat.astype(bf)], axis=1
    )
    wpack = np.ascontiguousarray(wpack)

    in_maps = [
        {
            "xb": np.ascontiguousarray(xb16[b]),
            "wpack": wpack,
        }
        for b in range(B)
    ]
    res = run_bass_kernel_spmd(
        nc, in_maps, core_ids=list(range(B)), trace=trace,
        trace_cores=list(range(B)) if trace else None,
    )
    kernel.last_result = res
    kernel.last_exec_time_ns = res.exec_time_ns
    return np.stack(
        [np.asarray(r["y"]).astype(np.float32) for r in res.results], axis=0
    )


def _numpy_fallback(inputs):
    """Faithful (but slow) mirror of the reference for unexpected inputs."""
    f32 = np.float32
    x = np.asarray(inputs["x"], f32)
    c = np.asarray(inputs["c"], f32)
    W1 = np.asarray(inputs["W1"], f32); W2 = np.asarray(inputs["W2"], f32)
    wt_w = np.asarray(inputs["wt_w"], f32); bsa = np.asarray(inputs["bsa"], f32)
    Wsa1 = np.asarray(inputs["Wsa1"], f32); Wsa2 = np.asarray(inputs["Wsa2"], f32)
    wsat_w = np.asarray(inputs["wsat_w"], f32)
    wsat_b = np.asarray(inputs["wsat_b"], f32); bsa1 = np.asarray(inputs["bsa1"], f32)
    pfn_w1 = np.asarray(inputs["pfn_w1"], f32); pfn_b1 = np.asarray(inputs["pfn_b1"], f32)
    pfn_w2 = np.asarray(inputs["pfn_w2"], f32); pfn_b2 = np.asarray(inputs["pfn_b2"], f32)
    ln_g = np.asarray(inputs["ln_g"], f32); ln_b = np.asarray(inputs["ln_b"], f32)
    Bs, Ls, _ = x.shape
    wx = x @ W1
    wq = c @ W2
    logits = (wx + wq[:, None, :] + bsa) @ wt_w
    m = logits.max(-1, keepdims=True)
    e = np.exp(logits - m)
    p = (e / e.sum(-1, keepdims=True))[..., None]
    h = x * p
    si = (h @ Wsa1) @ wsat_w
    sj = (h @ Wsa2) @ wsat_w
    const = bsa1 @ wsat_w + wsat_b
    colsum = np.zeros((Bs, Ls), f32)
    blk = 512
    for b in range(Bs):
        for i0 in range(0, Ls, blk):
            s = 1.0 / (1.0 + np.exp(-(si[b, i0 : i0 + blk, None] + sj[b, None, :] + const)))
            for r in range(s.shape[0]):
                s[r, i0 + r] = -np.inf
            sm = s.max(-1, keepdims=True)
            ee = np.exp(s - sm)
            colsum[b] += (ee / ee.sum(-1, keepdims=True)).sum(0)
    ui = x * colsum[..., None]
    yv = np.maximum(ui @ pfn_w1 + pfn_b1, 0.0)
    yv = yv @ pfn_w2 + pfn_b2 + ui
    mu = yv.mean(-1, keepdims=True)
    var = ((yv - mu) ** 2).mean(-1, keepdims=True)
    return ((yv - mu) / np.sqrt(var + 1e-6) * ln_g + ln_b).astype(f32)


def kernel(**inputs):
    x = np.asarray(inputs["x"], dtype=np.float32)
    pfn_w1 = np.asarray(inputs["pfn_w1"], dtype=np.float32)
    pfn_w2 = np.asarray(inputs["pfn_w2"], dtype=np.float32)

    fast_ok = (
        x.shape == (B, L, DX)
        and not np.any(np.asarray(inputs["pfn_b1"]))
        and not np.any(np.asarray(inputs["pfn_b2"]))
        and np.all(np.asarray(inputs["ln_g"]) == 1.0)
        and not np.any(np.asarray(inputs["ln_b"]))
    )
    if not fast_ok:
        return _numpy_fallback(inputs)

    trace = bool(int(os.environ.get("CSA_TRACE", "0")))
    return _run_device(x, pfn_w1, pfn_w2, trace=trace)


kernel.last_exec_time_ns = None
kernel.last_result = None


# revision 6
# speedup vs baseline: 1.8672x; 1.8672x over previous
"""Trainium2 Bass kernel for nn_CSATransformer_25778393710760.

Math: with the given parameters (all biases zero, ln identity) the module
reduces exactly to
    out = LN(relu(x @ pfn_w1) @ pfn_w2 + x)
(LayerNorm cancels the positive per-row colsum scale; see kernel_baseline).

Device kernel (per core, one batch example, L=4096 rows, D=128):
  - x is read ONCE from HBM, already transposed into SBUF via one whole-quad
    xbar DMA transpose (bf16, contiguous source).  No row-major copy of x is
    needed: the residual (and its row-centering) is folded into the PE as
    po[:,c,:] = xT_c @ C + relu(x @ W1)_c @ (W2 C),  C = I - J/128,
    so po = y - rowmean(y) exactly and LN reduces to po * rsqrt(mean(po^2)+eps).
  - row mapping: row = 2048*Q + 128*c + p (p = partition, quad Q, chunk c).
  - per quad (2048 rows) one PSUM tile [128,16,128] f32 (4 banks) holds both
    the mm1 staging (in the opposite pair's banks) and the final po chunks.
  - LN tail: ACT Square (1 op) -> DVE reduce -> ACT sqrt -> DVE recip ->
    DVE multiply (og, bf16) -> HWDGE store (bf16; host upcasts to fp32).
"""

import os
import numpy as np

B, L, DX = 8, 4096, 128
_NQ = 2            # quads per core
_C = 16            # chunks (rows per partition) per quad

_prog_cache = {}


def _build_program():
    import concourse.tile as tile
    from concourse import bacc, mybir
    from concourse.bass import ts

    f32 = mybir.dt.float32
    bf16 = mybir.dt.bfloat16
    AF = mybir.ActivationFunctionType
    OP = mybir.AluOpType
    AX = mybir.AxisListType

    nc = bacc.Bacc(None, target_bir_lowering=False)
    xb = nc.dram_tensor("xb", [L, DX], bf16, kind="ExternalInput")
    wpack = nc.dram_tensor("wpack", [DX, 3 * DX], bf16, kind="ExternalInput")
    y = nc.dram_tensor("y", [L, DX], bf16, kind="ExternalOutput")

    # row = 2048*q + 128*c + p: per quad, chunk c holds 128 consecutive rows.
    y_r = y.rearrange("(q c p) d -> q p c d", q=_NQ, p=128)

    with tile.TileContext(nc) as tc:
        with (
            tc.tile_pool(name="consts", bufs=1) as consts,
            tc.tile_pool(name="xt", bufs=2) as xt_pool,
            tc.tile_pool(name="y1s", bufs=2) as y1s_pool,
            tc.tile_pool(name="sqq", bufs=2) as sqq_pool,
            tc.tile_pool(name="og", bufs=2) as og_pool,
            tc.tile_pool(name="small", bufs=4) as small,
            tc.tile_pool(name="ps", bufs=2, space="PSUM") as ps,
        ):
            # ---- weights first on the scalar ring: gate matmuls
            wp = consts.tile([128, 3 * DX], bf16)
            nc.scalar.dma_start(out=wp, in_=wpack[:, :])
            w1_sb = wp[:, 0:128]
            w2c_sb = wp[:, 128:256]
            cmat_sb = wp[:, 256:384]

            eps = consts.tile([128, 1], f32)
            nc.vector.memset(eps, 1e-6)

            # ---- ACT table warms (Relu/Square/Sqrt) before data lands
            warm = consts.tile([128, 1], f32)
            nc.scalar.activation(out=warm, in_=eps, func=AF.Relu)
            nc.scalar.activation(out=warm, in_=eps, func=AF.Square)
            nc.scalar.activation(out=warm, in_=eps, func=AF.Sqrt, bias=eps)

            # ---- PE HAM warmup: real bf16 matmuls on the weight pack
            pewarm = ps.tile([128, _C, 128], f32, tag="T")
            for _ in range(4):
                nc.tensor.matmul(
                    pewarm[:, 0:3, :].rearrange("p c d -> p (c d)"),
                    lhsT=cmat_sb, rhs=wp[:, :], start=True, stop=True,
                )
            warmsink = consts.tile([128, 1], f32)
            nc.vector.tensor_copy(out=warmsink, in_=pewarm[:, 0, 0:1])

            for q in range(_NQ):
                # ---- load x transposed: xT[d, c, p] = x[2048q + 128c + p, d]
                xT = xt_pool.tile([128, _C, 128], bf16, tag="xT")
                nc.sync.dma_start_transpose(
                    out=xT.rearrange("d c p -> d (c p)"),
                    in_=xb[2048 * q : 2048 * (q + 1), :],
                )

                # ---- quad PSUM tile: mm1 stages y1 in the opposite pair's
                # banks, mm2 overwrites with po chunks.
                T = ps.tile([128, _C, 128], f32, tag="T")
                # y1 for chunk c lands at slot (8 + c) % 16
                for h in range(4):  # rhs chunk groups 0:4, 4:8, 8:12, 12:16
                    slot = (8 + 4 * h) % 16
                    nc.tensor.matmul(
                        T[:, slot : slot + 4, :].rearrange("p c d -> p (c d)"),
                        lhsT=w1_sb,
                        rhs=xT[:, ts(h, 4), :].rearrange("d c p -> d (c p)"),
                        start=True, stop=True,
                    )

                # ---- relu on the whole quad (y1 occupies all 16 slots)
                y1s = y1s_pool.tile([128, _C, 128], bf16, tag="y1s")
                nc.scalar.activation(
                    out=y1s.rearrange("e c p -> e (c p)"),
                    in_=T.rearrange("e c p -> e (c p)"),
                    func=AF.Relu,
                )

                # ---- po chunks: xc + relu@W2C, accumulated per chunk
                for c in range(_C):
                    nc.tensor.matmul(
                        T[:, c, :], lhsT=xT[:, c, :], rhs=cmat_sb,
                        start=True, stop=False,
                    )
                    nc.tensor.matmul(
                        T[:, c, :], lhsT=y1s[:, (8 + c) % 16, :], rhs=w2c_sb,
                        start=False, stop=True,
                    )

                # ---- LN tail: mean(po)=0 by construction
                sqq = sqq_pool.tile([128, _C, 128], bf16, tag="sqq")
                nc.scalar.activation(
                    out=sqq.rearrange("p c d -> p (c d)"),
                    in_=T.rearrange("p c d -> p (c d)"),
                    func=AF.Square,
                )
                ssq = small.tile([128, _C], f32, tag="ssq")
                nc.vector.tensor_reduce(out=ssq, in_=sqq, axis=AX.X, op=OP.add)
                std = small.tile([128, _C], f32, tag="std")
                nc.scalar.activation(out=std, in_=ssq, func=AF.Sqrt,
                                     scale=1.0 / 128.0, bias=eps)
                rstd = small.tile([128, _C], f32, tag="rstd")
                nc.vector.reciprocal(out=rstd, in_=std)

                og = og_pool.tile([128, _C, 128], bf16, tag="og")
                rb = rstd.to_broadcast([128, _C, 128])
                nc.vector.tensor_tensor(out=og, in0=T, in1=rb, op=OP.mult)

                nc.sync.dma_start(out=y_r[q], in_=og)
    nc.finalize()
    return nc


def _ensure_ntff_hook():
    """Register the axon NTFF profiling hook if the image lacks antenv.axon_hooks."""
    try:
        from antenv.axon_hooks import get_axon_ntff_profile_hook  # noqa: F401
        return
    except ImportError:
        pass
    import sys
    import types

    import antenv
    from trn_agent_boot.trn_boot import _ntff_profile_via_ctypes

    hook = _ntff_profile_via_ctypes("/opt/axon/libaxon_pjrt.so")
    mod = types.ModuleType("antenv.axon_hooks")
    mod._hook = hook
    mod.set_axon_ntff_profile_hook = lambda h: setattr(mod, "_hook", h)
    mod.get_axon_ntff_profile_hook = lambda: mod._hook
    sys.modules["antenv.axon_hooks"] = mod
    antenv.axon_hooks = mod


def _run_device(x, w1, w2, trace=False):
    import ml_dtypes
    import concourse.bass_utils as bass_utils
    from concourse.bass_utils import run_bass_kernel_spmd

    if trace:
        try:
            _ensure_ntff_hook()
            bass_utils.upload_artifacts = lambda tmpdir: str(tmpdir)
        except Exception as e:  # profiling is best-effort
            print(f"ntff hook unavailable ({e}); running without trace")
            trace = False

    if "prog" not in _prog_cache:
        _prog_cache["prog"] = _build_program()
    nc = _prog_cache["prog"]

    bf = ml_dtypes.bfloat16
    x = np.ascontiguousarray(x, dtype=np.float32)
    xb16 = x.astype(bf)

    w1c = np.ascontiguousarray(w1, dtype=np.float32)
    w2c = np.ascontiguousarray(w2, dtype=np.float32)
    cmat = np.eye(DX, dtype=np.float32) - np.float32(1.0 / DX)
    w2cc = (w2c @ cmat).astype(bf)
    wpack = np.concatenate(
        [w1c.astype(bf), w2cc, cmat.astype(bf)], axis=1
    )
    wpack = np.ascontiguousarray(wpack)

    in_maps = [
        {
            "xb": np.ascontiguousarray(xb16[b]),
            "wpack": wpack,
        }
        for b in range(B)
    ]
    res = run_bass_kernel_spmd(
        nc, in_maps, core_ids=list(range(B)), trace=trace,
        trace_cores=list(range(B)) if trace else None,
    )
    kernel.last_result = res
    kernel.last_exec_time_ns = res.exec_time_ns
    return np.stack(
        [np.asarray(r["y"]).astype(np.float32) for r in res.results], axis=0
    )


def _numpy_fallback(inputs):
    """Faithful (but slow) mirror of the reference for unexpected inputs."""
    f32 = np.float32
    x = np.asarray(inputs["x"], f32)
    c = np.asarray(inputs["c"], f32)
    W1 = np.asarray(inputs["W1"], f32); W2 = np.asarray(inputs["W2"], f32)
    wt_w = np.asarray(inputs["wt_w"], f32); bsa = np.asarray(inputs["bsa"], f32)
    Wsa1 = np.asarray(inputs["Wsa1"], f32); Wsa2 = np.asarray(inputs["Wsa2"], f32)
    wsat_w = np.asarray(inputs["wsat_w"], f32)
    wsat_b = np.asarray(inputs["wsat_b"], f32); bsa1 = np.asarray(inputs["bsa1"], f32)
    pfn_w1 = np.asarray(inputs["pfn_w1"], f32); pfn_b1 = np.asarray(inputs["pfn_b1"], f32)
    pfn_w2 = np.asarray(inputs["pfn_w2"], f32); pfn_b2 = np.asarray(inputs["pfn_b2"], f32)
    ln_g = np.asarray(inputs["ln_g"], f32); ln_b = np.asarray(inputs["ln_b"], f32)
    Bs, Ls, _ = x.shape
    wx = x @ W1
    wq = c @ W2
    logits = (wx + wq[:, None, :] + bsa) @ wt_w
    m = logits.max(-1, keepdims=True)
    e = np.exp(logits - m)
    p = (e / e.sum(-1, keepdims=True))[..., None]
    h = x * p
    si = (h @ Wsa1) @ wsat_w
    sj = (h @ Wsa2) @ wsat_w
    const = bsa1 @ wsat_w + wsat_b
    colsum = np.zeros((Bs, Ls), f32)
    blk = 512
    for b in range(Bs):
        for i0 in range(0, Ls, blk):
            s = 1.0 / (1.0 + np.exp(-(si[b, i0 : i0 + blk, None] + sj[b, None, :] + const)))
            for r in range(s.shape[0]):
                s[r, i0 + r] = -np.inf
            sm = s.max(-1, keepdims=True)
            ee = np.exp(s - sm)
            colsum[b] += (ee / ee.sum(-1, keepdims=True)).sum(0)
    ui = x * colsum[..., None]
    yv = np.maximum(ui @ pfn_w1 + pfn_b1, 0.0)
    yv = yv @ pfn_w2 + pfn_b2 + ui
    mu = yv.mean(-1, keepdims=True)
    var = ((yv - mu) ** 2).mean(-1, keepdims=True)
    return ((yv - mu) / np.sqrt(var + 1e-6) * ln_g + ln_b).astype(f32)


def kernel(**inputs):
    x = np.asarray(inputs["x"], dtype=np.float32)
    pfn_w1 = np.asarray(inputs["pfn_w1"], dtype=np.float32)
    pfn_w2 = np.asarray(inputs["pfn_w2"], dtype=np.float32)

    fast_ok = (
        x.shape == (B, L, DX)
        and not np.any(np.asarray(inputs["pfn_b1"]))
        and not np.any(np.asarray(inputs["pfn_b2"]))
        and np.all(np.asarray(inputs["ln_g"]) == 1.0)
        and not np.any(np.asarray(inputs["ln_b"]))
    )
    if not fast_ok:
        return _numpy_fallback(inputs)

    trace = bool(int(os.environ.get("CSA_TRACE", "0")))
    return _run_device(x, pfn_w1, pfn_w2, trace=trace)


kernel.last_exec_time_ns = None
kernel.last_result = None


# revision 7
# speedup vs baseline: 2.0272x; 1.0857x over previous
"""Trainium2 Bass kernel for nn_CSATransformer_25778393710760.

Math: with the given parameters (all biases zero, ln identity) the module
reduces exactly to
    out = LN(relu(x @ pfn_w1) @ pfn_w2 + x)
(LayerNorm cancels the positive per-row colsum scale; see kernel_baseline).

Device kernel (per core, one batch example, L=4096 rows, D=128):
  - x is read ONCE from HBM, transposed into SBUF via one whole-pair xbar
    DMA transpose (bf16, contiguous source).  No row-major copy of x is
    needed: the residual (and its row-centering) is folded into the PE as
    po[:,c,:] = xT_c @ C + relu(x @ W1)_c @ (W2 C),  C = I - J/128,
    so po = y - rowmean(y) exactly and LN reduces to po * rsqrt(mean(po^2)+eps).
  - row mapping: row = 1024*P + 8*p + c (pair P, partition p, chunk c=0..7).
    mm2's stationary operands are stride-8 column windows of y1s/xT, which
    makes each og partition hold 8 consecutive rows -> 2KB-contiguous store
    descriptors at line rate.
  - per pair (1024 rows) one PSUM tile [128,8,128] f32 (2 banks) stages the
    mm1 output in-place (relu consumes it before mm2 overwrites with po).
  - LN tail: ACT Square -> DVE reduce -> ACT sqrt -> DVE recip ->
    DVE multiply (og, bf16) -> HWDGE store (bf16; host upcasts to fp32).
"""

import os
import numpy as np

B, L, DX = 8, 4096, 128
_NP = 4            # pairs per core
_C = 8             # chunks (rows per partition) per pair

_prog_cache = {}


def _build_program():
    import concourse.tile as tile
    from concourse import bacc, mybir
    from concourse.bass import ts

    f32 = mybir.dt.float32
    bf16 = mybir.dt.bfloat16
    AF = mybir.ActivationFunctionType
    OP = mybir.AluOpType
    AX = mybir.AxisListType

    nc = bacc.Bacc(None, target_bir_lowering=False)
    xb = nc.dram_tensor("xb", [L, DX], bf16, kind="ExternalInput")
    wpack = nc.dram_tensor("wpack", [DX, 3 * DX], bf16, kind="ExternalInput")
    y = nc.dram_tensor("y", [L, DX], bf16, kind="ExternalOutput")

    # row = 1024*P + 8*p + c: per pair, partition p holds 8 consecutive rows.
    y_r = y.rearrange("(P p c) d -> P p c d", P=_NP, p=128)

    with tile.TileContext(nc) as tc:
        with (
            tc.tile_pool(name="consts", bufs=1) as consts,
            tc.tile_pool(name="xt", bufs=3) as xt_pool,
            tc.tile_pool(name="y1s", bufs=3) as y1s_pool,
            tc.tile_pool(name="sqq", bufs=3) as sqq_pool,
            tc.tile_pool(name="og", bufs=3) as og_pool,
            tc.tile_pool(name="small", bufs=8) as small,
            tc.tile_pool(name="ps", bufs=4, space="PSUM") as ps,
        ):
            # ---- weights first on the scalar ring: gate matmuls
            wp = consts.tile([128, 3 * DX], bf16)
            nc.scalar.dma_start(out=wp, in_=wpack[:, :])
            w1_sb = wp[:, 0:128]
            w2c_sb = wp[:, 128:256]
            cmat_sb = wp[:, 256:384]

            eps = consts.tile([128, 1], f32)
            nc.vector.memset(eps, 1e-6)

            # ---- ACT table warms (Relu/Square/Sqrt) before data lands
            warm = consts.tile([128, 1], f32)
            nc.scalar.activation(out=warm, in_=eps, func=AF.Relu)
            nc.scalar.activation(out=warm, in_=eps, func=AF.Square)
            nc.scalar.activation(out=warm, in_=eps, func=AF.Sqrt, bias=eps)

            # ---- PE HAM warmup: real bf16 matmuls on the weight pack
            pewarm = ps.tile([128, _C, 128], f32, tag="T")
            for _ in range(4):
                nc.tensor.matmul(
                    pewarm[:, 0:3, :].rearrange("p c d -> p (c d)"),
                    lhsT=cmat_sb, rhs=wp[:, :], start=True, stop=True,
                )
            warmsink = consts.tile([128, 1], f32)
            nc.vector.tensor_copy(out=warmsink, in_=pewarm[:, 0, 0:1])

            for q in range(_NP):
                # ---- load x transposed: xT[d, j] = x[1024q + j, d]
                xT = xt_pool.tile([128, _C, 128], bf16, tag="xT")
                nc.sync.dma_start_transpose(
                    out=xT.rearrange("d a b -> d (a b)"),
                    in_=xb[1024 * q : 1024 * (q + 1), :],
                )
                # stride-8 view: xT_r[d, c, p] = xT[d, 8p + c] = x row 8p+c
                xT_r = xT.rearrange("d a b -> d (a b)").rearrange(
                    "d (p c) -> d c p", c=_C
                )

                # ---- pair PSUM tile: mm1 stages y1 in place, mm2 overwrites
                T = ps.tile([128, _C, 128], f32, tag="T")
                for h in range(2):
                    nc.tensor.matmul(
                        T[:, ts(h, 4), :].rearrange("p c d -> p (c d)"),
                        lhsT=w1_sb,
                        rhs=xT.rearrange("d a b -> d (a b)")[:, ts(h, 512)],
                        start=True, stop=True,
                    )

                # ---- relu on the whole pair (y1 occupies all 8 slots)
                y1s = y1s_pool.tile([128, _C, 128], bf16, tag="y1s")
                nc.scalar.activation(
                    out=y1s.rearrange("e a b -> e (a b)"),
                    in_=T.rearrange("e a b -> e (a b)"),
                    func=AF.Relu,
                )
                y1s_r = y1s.rearrange("e a b -> e (a b)").rearrange(
                    "e (p c) -> e c p", c=_C
                )

                # ---- po chunks: xc + relu@W2C; chunk c = rows {8p + c}
                for c in range(_C):
                    nc.tensor.matmul(
                        T[:, c, :], lhsT=xT_r[:, c, :], rhs=cmat_sb,
                        start=True, stop=False,
                    )
                    nc.tensor.matmul(
                        T[:, c, :], lhsT=y1s_r[:, c, :], rhs=w2c_sb,
                        start=False, stop=True,
                    )

                # ---- LN tail: mean(po)=0 by construction
                sqq = sqq_pool.tile([128, _C, 128], bf16, tag="sqq")
                nc.scalar.activation(
                    out=sqq.rearrange("p c d -> p (c d)"),
                    in_=T.rearrange("p c d -> p (c d)"),
                    func=AF.Square,
                )
                ssq = small.tile([128, _C], f32, tag="ssq")
                nc.vector.tensor_reduce(out=ssq, in_=sqq, axis=AX.X, op=OP.add)
                std = small.tile([128, _C], f32, tag="std")
                nc.scalar.activation(out=std, in_=ssq, func=AF.Sqrt,
                                     scale=1.0 / 128.0, bias=eps)
                rstd = small.tile([128, _C], f32, tag="rstd")
                nc.vector.reciprocal(out=rstd, in_=std)

                og = og_pool.tile([128, _C, 128], bf16, tag="og")
                rb = rstd.to_broadcast([128, _C, 128])
                nc.vector.tensor_tensor(out=og, in0=T, in1=rb, op=OP.mult)

                nc.sync.dma_start(out=y_r[q], in_=og)
    nc.finalize()
    return nc


def _ensure_ntff_hook():
    """Register the axon NTFF profiling hook if the image lacks antenv.axon_hooks."""
    try:
        from antenv.axon_hooks import get_axon_ntff_profile_hook  # noqa: F401
        return
    except ImportError:
        pass
    import sys
    import types

    import antenv
    from trn_agent_boot.trn_boot import _ntff_profile_via_ctypes

    hook = _ntff_profile_via_ctypes("/opt/axon/libaxon_pjrt.so")
    mod = types.ModuleType("antenv.axon_hooks")
    mod._hook = hook
    mod.set_axon_ntff_profile_hook = lambda h: setattr(mod, "_hook", h)
    mod.get_axon_ntff_profile_hook = lambda: mod._hook
    sys.modules["antenv.axon_hooks"] = mod
    antenv.axon_hooks = mod


def _run_device(x, w1, w2, trace=False):
    import ml_dtypes
    import concourse.bass_utils as bass_utils
    from concourse.bass_utils import run_bass_kernel_spmd

    if trace:
        try:
            _ensure_ntff_hook()
            bass_utils.upload_artifacts = lambda tmpdir: str(tmpdir)
        except Exception as e:  # profiling is best-effort
            print(f"ntff hook unavailable ({e}); running without trace")
            trace = False

    if "prog" not in _prog_cache:
        _prog_cache["prog"] = _build_program()
    nc = _prog_cache["prog"]

    bf = ml_dtypes.bfloat16
    x = np.ascontiguousarray(x, dtype=np.float32)
    xb16 = x.astype(bf)

    w1c = np.ascontiguousarray(w1, dtype=np.float32)
    w2c = np.ascontiguousarray(w2, dtype=np.float32)
    cmat = np.eye(DX, dtype=np.float32) - np.float32(1.0 / DX)
    w2cc = (w2c @ cmat).astype(bf)
    wpack = np.concatenate(
        [w1c.astype(bf), w2cc, cmat.astype(bf)], axis=1
    )
    wpack = np.ascontiguousarray(wpack)

    in_maps = [
        {
            "xb": np.ascontiguousarray(xb16[b]),
            "wpack": wpack,
        }
        for b in range(B)
    ]
    res = run_bass_kernel_spmd(
        nc, in_maps, core_ids=list(range(B)), trace=trace,
        trace_cores=list(range(B)) if trace else None,
    )
    kernel.last_result = res
    kernel.last_exec_time_ns = res.exec_time_ns
    return np.stack(
        [np.asarray(r["y"]).astype(np.float32) for r in res.results], axis=0
    )


def _numpy_fallback(inputs):
    """Faithful (but slow) mirror of the reference for unexpected inputs."""
    f32 = np.float32
    x = np.asarray(inputs["x"], f32)
    c = np.asarray(inputs["c"], f32)
    W1 = np.asarray(inputs["W1"], f32); W2 = np.asarray(inputs["W2"], f32)
    wt_w = np.asarray(inputs["wt_w"], f32); bsa = np.asarray(inputs["bsa"], f32)
    Wsa1 = np.asarray(inputs["Wsa1"], f32); Wsa2 = np.asarray(inputs["Wsa2"], f32)
    wsat_w = np.asarray(inputs["wsat_w"], f32)
    wsat_b = np.asarray(inputs["wsat_b"], f32); bsa1 = np.asarray(inputs["bsa1"], f32)
    pfn_w1 = np.asarray(inputs["pfn_w1"], f32); pfn_b1 = np.asarray(inputs["pfn_b1"], f32)
    pfn_w2 = np.asarray(inputs["pfn_w2"], f32); pfn_b2 = np.asarray(inputs["pfn_b2"], f32)
    ln_g = np.asarray(inputs["ln_g"], f32); ln_b = np.asarray(inputs["ln_b"], f32)
    Bs, Ls, _ = x.shape
    wx = x @ W1
    wq = c @ W2
    logits = (wx + wq[:, None, :] + bsa) @ wt_w
    m = logits.max(-1, keepdims=True)
    e = np.exp(logits - m)
    p = (e / e.sum(-1, keepdims=True))[..., None]
    h = x * p
    si = (h @ Wsa1) @ wsat_w
    sj = (h @ Wsa2) @ wsat_w
    const = bsa1 @ wsat_w + wsat_b
    colsum = np.zeros((Bs, Ls), f32)
    blk = 512
    for b in range(Bs):
        for i0 in range(0, Ls, blk):
            s = 1.0 / (1.0 + np.exp(-(si[b, i0 : i0 + blk, None] + sj[b, None, :] + const)))
            for r in range(s.shape[0]):
                s[r, i0 + r] = -np.inf
            sm = s.max(-1, keepdims=True)
            ee = np.exp(s - sm)
            colsum[b] += (ee / ee.sum(-1, keepdims=True)).sum(0)
    ui = x * colsum[..., None]
    yv = np.maximum(ui @ pfn_w1 + pfn_b1, 0.0)
    yv = yv @ pfn_w2 + pfn_b2 + ui
    mu = yv.mean(-1, keepdims=True)
    var = ((yv - mu) ** 2).mean(-1, keepdims=True)
    return ((yv - mu) / np.sqrt(var + 1e-6) * ln_g + ln_b).astype(f32)


def kernel(**inputs):
    x = np.asarray(inputs["x"], dtype=np.float32)
    pfn_w1 = np.asarray(inputs["pfn_w1"], dtype=np.float32)
    pfn_w2 = np.asarray(inputs["pfn_w2"], dtype=np.float32)

    fast_ok = (
        x.shape == (B, L, DX)
        and not np.any(np.asarray(inputs["pfn_b1"]))
        and not np.any(np.asarray(inputs["pfn_b2"]))
        and np.all(np.asarray(inputs["ln_g"]) == 1.0)
        and not np.any(np.asarray(inputs["ln_b"]))
    )
    if not fast_ok:
        return _numpy_fallback(inputs)

    trace = bool(int(os.environ.get("CSA_TRACE", "0")))
    return _run_device(x, pfn_w1, pfn_w2, trace=trace)


kernel.last_exec_time_ns = None
kernel.last_result = None


# revision 9
# speedup vs baseline: 2.0361x; 1.0044x over previous
"""Trainium2 Bass kernel for nn_CSATransformer_25778393710760.

Math: with the given parameters (all biases zero, ln identity) the module
reduces exactly to
    out = LN(relu(x @ pfn_w1) @ pfn_w2 + x)
(LayerNorm cancels the positive per-row colsum scale; see kernel_baseline).

Device kernel (per core, one batch example, L=4096 rows, D=128):
  - x is read ONCE from HBM, transposed into SBUF via one whole-pair xbar
    DMA transpose (bf16, contiguous source).  No row-major copy of x is
    needed: the residual (and its row-centering) is folded into the PE as
    po[:,c,:] = xT_c @ C + relu(x @ W1)_c @ (W2 C),  C = I - J/128,
    so po = y - rowmean(y) exactly and LN reduces to po * rsqrt(mean(po^2)+eps).
  - row mapping: row = 1024*P + 8*p + c (pair P, partition p, chunk c=0..7).
    mm2's stationary operands are stride-8 column windows of y1s/xT, which
    makes each og partition hold 8 consecutive rows -> 2KB-contiguous store
    descriptors at line rate.
  - per pair (1024 rows) one PSUM tile [128,8,128] f32 (2 banks) stages the
    mm1 output in-place (relu consumes it before mm2 overwrites with po).
  - LN tail: ACT Square -> DVE reduce -> ACT sqrt -> DVE recip ->
    DVE multiply (og, bf16) -> HWDGE store (bf16; host upcasts to fp32).
"""

import os
import numpy as np

B, L, DX = 8, 4096, 128
_NP = 4            # pairs per core
_C = 8             # chunks (rows per partition) per pair

_prog_cache = {}


def _build_program():
    import concourse.tile as tile
    from concourse import bacc, mybir
    from concourse.bass import ts

    f32 = mybir.dt.float32
    bf16 = mybir.dt.bfloat16
    AF = mybir.ActivationFunctionType
    OP = mybir.AluOpType
    AX = mybir.AxisListType

    nc = bacc.Bacc(None, target_bir_lowering=False)
    xb = nc.dram_tensor("xb", [L, DX], bf16, kind="ExternalInput")
    wpack = nc.dram_tensor("wpack", [DX, 3 * DX], bf16, kind="ExternalInput")
    y = nc.dram_tensor("y", [L, DX], bf16, kind="ExternalOutput")

    # row = 1024*P + 8*p + c: per pair, partition p holds 8 consecutive rows.
    y_r = y.rearrange("(P p c) d -> P p c d", P=_NP, p=128)

    with tile.TileContext(nc) as tc:
        with (
            tc.tile_pool(name="consts", bufs=1) as consts,
            tc.tile_pool(name="xt", bufs=4) as xt_pool,
            tc.tile_pool(name="y1s", bufs=3) as y1s_pool,
            tc.tile_pool(name="sqq", bufs=3) as sqq_pool,
            tc.tile_pool(name="og", bufs=3) as og_pool,
            tc.tile_pool(name="small", bufs=8) as small,
            tc.tile_pool(name="ps", bufs=4, space="PSUM") as ps,
        ):
            # ---- weights on the sync ring (keeps ACT free for table loads)
            wp = consts.tile([128, 3 * DX], bf16)
            nc.sync.dma_start(out=wp, in_=wpack[:, :])
            w1_sb = wp[:, 0:128]
            w2c_sb = wp[:, 128:256]
            cmat_sb = wp[:, 256:384]

            # ---- all 4 transposed loads up front: overlap warmup + each other
            xTs = []
            for q in range(_NP):
                xT = xt_pool.tile([128, _C, 128], bf16, tag=f"xT{q}")
                nc.sync.dma_start_transpose(
                    out=xT.rearrange("d a b -> d (a b)"),
                    in_=xb[1024 * q : 1024 * (q + 1), :],
                )
                xTs.append(xT)

            eps = consts.tile([128, 1], f32)
            nc.vector.memset(eps, 1e-6)

            # ---- ACT table warms (Relu/Square/Sqrt) before data lands
            warm = consts.tile([128, 1], f32)
            nc.scalar.activation(out=warm, in_=eps, func=AF.Relu)
            nc.scalar.activation(out=warm, in_=eps, func=AF.Square)
            nc.scalar.activation(out=warm, in_=eps, func=AF.Sqrt, bias=eps)

            # ---- PE HAM warmup: ~4us of matmuls so the clock gate opens
            # before real work; overlaps the transpose DMAs' latency.
            pewarm = ps.tile([128, _C, 128], f32, tag="T")
            for _ in range(10):
                nc.tensor.matmul(
                    pewarm[:, 0:3, :].rearrange("p c d -> p (c d)"),
                    lhsT=cmat_sb, rhs=wp[:, :], start=True, stop=True,
                )
            warmsink = consts.tile([128, 1], f32)
            nc.vector.tensor_copy(out=warmsink, in_=pewarm[:, 0, 0:1])

            for q in range(_NP):
                xT = xTs[q]
                # stride-8 view: xT_r[d, c, p] = xT[d, 8p + c] = x row 8p+c
                xT_r = xT.rearrange("d a b -> d (a b)").rearrange(
                    "d (p c) -> d c p", c=_C
                )

                # ---- pair PSUM tile: mm1 stages y1 in place, mm2 overwrites
                T = ps.tile([128, _C, 128], f32, tag="T")
                for h in range(2):
                    nc.tensor.matmul(
                        T[:, ts(h, 4), :].rearrange("p c d -> p (c d)"),
                        lhsT=w1_sb,
                        rhs=xT.rearrange("d a b -> d (a b)")[:, ts(h, 512)],
                        start=True, stop=True,
                    )

                # ---- relu on the whole pair (y1 occupies all 8 slots)
                y1s = y1s_pool.tile([128, _C, 128], bf16, tag="y1s")
                nc.scalar.activation(
                    out=y1s.rearrange("e a b -> e (a b)"),
                    in_=T.rearrange("e a b -> e (a b)"),
                    func=AF.Relu,
                )
                y1s_r = y1s.rearrange("e a b -> e (a b)").rearrange(
                    "e (p c) -> e c p", c=_C
                )

                # ---- po chunks: xc + relu@W2C; chunk c = rows {8p + c}
                for c in range(_C):
                    nc.tensor.matmul(
                        T[:, c, :], lhsT=xT_r[:, c, :], rhs=cmat_sb,
                        start=True, stop=False,
                    )
                    nc.tensor.matmul(
                        T[:, c, :], lhsT=y1s_r[:, c, :], rhs=w2c_sb,
                        start=False, stop=True,
                    )

                # ---- LN tail: mean(po)=0 by construction
                sqq = sqq_pool.tile([128, _C, 128], bf16, tag="sqq")
                nc.scalar.activation(
                    out=sqq.rearrange("p c d -> p (c d)"),
                    in_=T.rearrange("p c d -> p (c d)"),
                    func=AF.Square,
                )
                ssq = small.tile([128, _C], f32, tag="ssq")
                nc.vector.tensor_reduce(out=ssq, in_=sqq, axis=AX.X, op=OP.add)
                std = small.tile([128, _C], f32, tag="std")
                nc.scalar.activation(out=std, in_=ssq, func=AF.Sqrt,
                                     scale=1.0 / 128.0, bias=eps)
                rstd = small.tile([128, _C], f32, tag="rstd")
                nc.vector.reciprocal(out=rstd, in_=std)

                og = og_pool.tile([128, _C, 128], bf16, tag="og")
                rb = rstd.to_broadcast([128, _C, 128])
                nc.vector.tensor_tensor(out=og, in0=T, in1=rb, op=OP.mult)

                nc.sync.dma_start(out=y_r[q], in_=og)
    nc.finalize()
    return nc


def _ensure_ntff_hook():
    """Register the axon NTFF profiling hook if the image lacks antenv.axon_hooks."""
    try:
        from antenv.axon_hooks import get_axon_ntff_profile_hook  # noqa: F401
        return
    except ImportError:
        pass
    import sys
    import types

    import antenv
    from trn_agent_boot.trn_boot import _ntff_profile_via_ctypes

    hook = _ntff_profile_via_ctypes("/opt/axon/libaxon_pjrt.so")
    mod = types.ModuleType("antenv.axon_hooks")
    mod._hook = hook
    mod.set_axon_ntff_profile_hook = lambda h: setattr(mod, "_hook", h)
    mod.get_axon_ntff_profile_hook = lambda: mod._hook
    sys.modules["antenv.axon_hooks"] = mod
    antenv.axon_hooks = mod


def _run_device(x, w1, w2, trace=False):
    import ml_dtypes
    import concourse.bass_utils as bass_utils
    from concourse.bass_utils import run_bass_kernel_spmd

    if trace:
        try:
            _ensure_ntff_hook()
            bass_utils.upload_artifacts = lambda tmpdir: str(tmpdir)
        except Exception as e:  # profiling is best-effort
            print(f"ntff hook unavailable ({e}); running without trace")
            trace = False

    if "prog" not in _prog_cache:
        _prog_cache["prog"] = _build_program()
    nc = _prog_cache["prog"]

    bf = ml_dtypes.bfloat16
    x = np.ascontiguousarray(x, dtype=np.float32)
    xb16 = x.astype(bf)

    w1c = np.ascontiguousarray(w1, dtype=np.float32)
    w2c = np.ascontiguousarray(w2, dtype=np.float32)
    cmat = np.eye(DX, dtype=np.float32) - np.float32(1.0 / DX)
    w2cc = (w2c @ cmat).astype(bf)
    wpack = np.concatenate(
        [w1c.astype(bf), w2cc, cmat.astype(bf)], axis=1
    )
    wpack = np.ascontiguousarray(wpack)

    in_maps = [
        {
            "xb": np.ascontiguousarray(xb16[b]),
            "wpack": wpack,
        }
        for b in range(B)
    ]
    res = run_bass_kernel_spmd(
        nc, in_maps, core_ids=list(range(B)), trace=trace,
        trace_cores=list(range(B)) if trace else None,
    )
    kernel.last_result = res
    kernel.last_exec_time_ns = res.exec_time_ns
    return np.stack(
        [np.asarray(r["y"]).astype(np.float32) for r in res.results], axis=0
    )


def _numpy_fallback(inputs):
    """Faithful (but slow) mirror of the reference for unexpected inputs."""
    f32 = np.float32
    x = np.asarray(inputs["x"], f32)
    c = np.asarray(inputs["c"], f32)
    W1 = np.asarray(inputs["W1"], f32); W2 = np.asarray(inputs["W2"], f32)
    wt_w = np.asarray(inputs["wt_w"], f32); bsa = np.asarray(inputs["bsa"], f32)
    Wsa1 = np.asarray(inputs["Wsa1"], f32); Wsa2 = np.asarray(inputs["Wsa2"], f32)
    wsat_w = np.asarray(inputs["wsat_w"], f32)
    wsat_b = np.asarray(inputs["wsat_b"], f32); bsa1 = np.asarray(inputs["bsa1"], f32)
    pfn_w1 = np.asarray(inputs["pfn_w1"], f32); pfn_b1 = np.asarray(inputs["pfn_b1"], f32)
    pfn_w2 = np.asarray(inputs["pfn_w2"], f32); pfn_b2 = np.asarray(inputs["pfn_b2"], f32)
    ln_g = np.asarray(inputs["ln_g"], f32); ln_b = np.asarray(inputs["ln_b"], f32)
    Bs, Ls, _ = x.shape
    wx = x @ W1
    wq = c @ W2
    logits = (wx + wq[:, None, :] + bsa) @ wt_w
    m = logits.max(-1, keepdims=True)
    e = np.exp(logits - m)
    p = (e / e.sum(-1, keepdims=True))[..., None]
    h = x * p
    si = (h @ Wsa1) @ wsat_w
    sj = (h @ Wsa2) @ wsat_w
    const = bsa1 @ wsat_w + wsat_b
    colsum = np.zeros((Bs, Ls), f32)
    blk = 512
    for b in range(Bs):
        for i0 in range(0, Ls, blk):
            s = 1.0 / (1.0 + np.exp(-(si[b, i0 : i0 + blk, None] + sj[b, None, :] + const)))
            for r in range(s.shape[0]):
                s[r, i0 + r] = -np.inf
            sm = s.max(-1, keepdims=True)
            ee = np.exp(s - sm)
            colsum[b] += (ee / ee.sum(-1, keepdims=True)).sum(0)
    ui = x * colsum[..., None]
    yv = np.maximum(ui @ pfn_w1 + pfn_b1, 0.0)
    yv = yv @ pfn_w2 + pfn_b2 + ui
    mu = yv.mean(-1, keepdims=True)
    var = ((yv - mu) ** 2).mean(-1, keepdims=True)
    return ((yv - mu) / np.sqrt(var + 1e-6) * ln_g + ln_b).astype(f32)


def kernel(**inputs):
    x = np.asarray(inputs["x"], dtype=np.float32)
    pfn_w1 = np.asarray(inputs["pfn_w1"], dtype=np.float32)
    pfn_w2 = np.asarray(inputs["pfn_w2"], dtype=np.float32)

    fast_ok = (
        x.shape == (B, L, DX)
        and not np.any(np.asarray(inputs["pfn_b1"]))
        and not np.any(np.asarray(inputs["pfn_b2"]))
        and np.all(np.asarray(inputs["ln_g"]) == 1.0)
        and not np.any(np.asarray(inputs["ln_b"]))
    )
    if not fast_ok:
        return _numpy_fallback(inputs)

    trace = bool(int(os.environ.get("CSA_TRACE", "0")))
    return _run_device(x, pfn_w1, pfn_w2, trace=trace)


kernel.last_exec_time_ns = None
kernel.last_result = None
